# revision 1
# baseline (speedup 1.0000x reference)
import numpy as np
import jax
import jax.numpy as jnp
from functools import partial

# nn_Block_89283780149784 — spiking transformer block.
# Data-parallel over B across the 8 NeuronCores (jax pmap); all LIF
# recurrences are over T, BN is eval-mode affine, attention is per (b,h).

T, B, C, N, H = 10, 128, 512, 16, 16
D = C // H
HID = 2048
TAU, THR, SCALE, ALPHA_MIX = 2.0, 1.0, 0.25, 0.5
NCORES = 8


def _lif(x):
    def step(mem, inp):
        mem = mem + (inp - mem) / TAU
        s = (mem > THR).astype(x.dtype)
        mem = mem * (1.0 - s)
        return mem, s
    return jax.lax.scan(step, jnp.zeros_like(x[0]), x)[1]


def _lif_step(mem, inp):
    mem = mem + (inp - mem) / TAU
    s = (mem > THR).astype(inp.dtype)
    mem = mem * (1.0 - s)
    return s, mem


def _bn(x, p):
    g, b, m, v = p
    inv = g * jax.lax.rsqrt(v + 1e-5)
    return (x - m[None, None, :, None]) * inv[None, None, :, None] + b[None, None, :, None]


def _conv1x1(x, W):
    return jnp.einsum('oc,tbcn->tbon', W, x)


def _block(x, Wq, Wk, Wv, Wproj, bn_q, bn_k, bn_v, bn_proj, ti_w, ti_b,
           W1, b1, bn1, W2, b2, bn2):
    Bl = x.shape[1]

    def qkv(W, p):
        y = _lif(_bn(_conv1x1(x, W), p))
        return y.reshape(T, Bl, N, H, D).transpose(0, 1, 3, 2, 4)

    q = qkv(Wq, bn_q)
    k = qkv(Wk, bn_k)
    v = qkv(Wv, bn_v)

    out0 = (q[0] @ jnp.swapaxes(k[0], -2, -1) * SCALE) @ v[0]

    def step(carry, qkv_t):
        q_ti, mem1, mem2 = carry
        qt, kt, vt = qkv_t
        c = q_ti.reshape(Bl * H, N, D)
        c = jax.lax.conv_general_dilated(
            c, ti_w, (1,), ((2, 2),), dimension_numbers=('NCH', 'OIH', 'NCH'))
        c = c + ti_b[None, :, None]
        c = c.reshape(Bl, H, N, D)
        s1, mem1 = _lif_step(mem1, c)
        mix = s1 * ALPHA_MIX + qt * (1.0 - ALPHA_MIX)
        s2, mem2 = _lif_step(mem2, mix)
        attn = (s2 @ jnp.swapaxes(kt, -2, -1) * SCALE) @ vt
        return (s2, mem1, mem2), attn

    init = (q[0], jnp.zeros_like(q[0]), jnp.zeros_like(q[0]))
    _, outs = jax.lax.scan(step, init, (q[1:], k[1:], v[1:]))
    out = jnp.concatenate([out0[None], outs], axis=0)

    y = jnp.swapaxes(out, 3, 4).reshape(T, Bl, C, N)
    y = _lif(y)
    y = _lif(_bn(_conv1x1(y, Wproj), bn_proj))
    x1 = x + y
    h = _lif(_bn(_conv1x1(x1, W1) + b1[None, None, :, None], bn1))
    m = _lif(_bn(_conv1x1(h, W2) + b2[None, None, :, None], bn2))
    return x1 + m


@partial(jax.pmap, axis_name='i',
         in_axes=(0,) + (None,) * 16,
         static_broadcasted_argnums=())
def _pmapped(x, Wq, Wk, Wv, Wproj, bn_q, bn_k, bn_v, bn_proj, ti_w, ti_b,
             W1, b1, bn1, W2, b2, bn2):
    return _block(x, Wq, Wk, Wv, Wproj, bn_q, bn_k, bn_v, bn_proj, ti_w, ti_b,
                  W1, b1, bn1, W2, b2, bn2)


def kernel(x, Wq, Wk, Wv, Wproj, bn_q, bn_k, bn_v, bn_proj, ti_w, ti_b,
           W1, b1, bn1, W2, b2, bn2):
    # shard batch over the 8 cores: (T, B, C, N) -> (8, T, B/8, C, N)
    xs = np.ascontiguousarray(
        np.asarray(x).reshape(T, NCORES, B // NCORES, C, N).transpose(1, 0, 2, 3, 4))
    out = _pmapped(jnp.asarray(xs), Wq, Wk, Wv, Wproj,
                   bn_q, bn_k, bn_v, bn_proj, ti_w, ti_b,
                   W1, b1, bn1, W2, b2, bn2)
    out = np.asarray(out)  # (8, T, B/8, C, N)
    return np.ascontiguousarray(
        out.transpose(1, 0, 2, 3, 4).reshape(T, B, C, N)).astype(np.float32)
